# revision 9
# baseline (speedup 1.0000x reference)
"""Trainium2 Bass kernel for BiDAF-style bidirectional attention (v4).

Reference computation (per batch element n; M=1 folded away):
    s[i,j]  = h[i].w_h + u[j].w_u + (h[i]*u[j]).w_hu + b      [JX, JQ]
    a_u     = softmax_j(s);     u_a[i] = sum_j a_u[i,j] u[j]   (c2q)
    a_h     = softmax_i(max_j s);  h_a = sum_i a_h[i] h[i]     (q2c)
    out     = concat(h, u_a, h*u_a, h*h_a)                     [JX, 4D]

Sharding: data-parallel over batch N=8, one NeuronCore per batch element.
alpha_b drops out (both softmaxes are shift-invariant); accepted but unused.

v4 design notes (on top of v3's bf16-I/O + host-folded weights):
  - Input DMAs chained on the Sync HW queue in priority order (hT-b0 gates
    the first scores; h halves land last).  aux on the Scalar queue, uwu on
    GpSimd.  Total input 2.25MB at the ~300GB/s read path.
  - Output slab 0 (h passthrough) is a pure DRAM->DRAM DMA from the input
    tensor -- it streams during the otherwise write-idle input phase.
  - 5-matmul bf16 warmup => HAM clock gate opens ~11.5us, so scores/c2q/q2c
    matmuls run at 2.4GHz.
  - o2 (= u_a/z PSUM evict) split Scalar ACT x6 / GpSimd tensor_scalar x2;
    o3 = o2*h all-bf16 2x TT on DVE only (DVE+GpSimd share SBUF ports --
    big muls never run on both).  o4 computed transposed from the resident
    hT via 4x-mode DVE tensor_scalar; host un-transposes slab 3.
  - zsum over j via tiny N=1 PE matmuls (ET_t^T @ ones) instead of DVE
    reduces; only the j-max reduces remain on DVE.
"""

import numpy as np

N_B, M_B, JX, JQ, D = 8, 1, 1024, 128, 512
P = 128
NT = JX // P    # 8 i-tiles
KC = D // P     # 4 d-chunks
IB = 512        # i-block width for score matmuls
NB = JX // IB   # 2 blocks
TPB = NT // NB  # 4 tiles per block

_CACHE = {}


def _build_program():
    from contextlib import ExitStack

    import concourse.bass as bass
    import concourse.tile as tile
    from concourse import bacc, mybir
    from concourse.masks import make_identity

    f32 = mybir.dt.float32
    bf16 = mybir.dt.bfloat16
    EXP = mybir.ActivationFunctionType.Exp
    AX = mybir.AxisListType.X
    ds = bass.ds

    nc = bacc.Bacc("TRN2", target_bir_lowering=False, debug=False, num_devices=8)
    h_d = nc.dram_tensor("hrows", [P, NT * D], bf16, kind="ExternalInput").ap()
    hT_d = nc.dram_tensor("hT", [P, KC * JX], bf16, kind="ExternalInput").ap()
    aux_d = nc.dram_tensor("aux", [P, 2 * D], bf16, kind="ExternalInput").ap()
    uwu_d = nc.dram_tensor("uwu", [P, 1], f32, kind="ExternalInput").ap()
    # slabs: 0=h rows, 1=u_a rows, 2=h*u_a rows, 3=o4T chunk-major
    out_d = nc.dram_tensor("out", [P, 4 * NT * D], bf16, kind="ExternalOutput").ap()

    with tile.TileContext(nc) as tc, ExitStack() as ctx:
        consts = ctx.enter_context(tc.tile_pool(name="consts", bufs=1))
        stage = ctx.enter_context(tc.tile_pool(name="stage", bufs=1))
        # PSUM budget (8 banks): acc=1, s0=2, tp=2, ua=2, hap=1
        ps = ctx.enter_context(tc.tile_pool(name="ps", bufs=2, space="PSUM"))

        # ---- input DMAs.  Sync queue carries the big loads in priority
        # order (transfers execute in queue order, so hT-b0 is never
        # starved); aux rides the Scalar queue, uwu the GpSimd queue. ----
        hT = consts.tile([P, KC * JX], bf16)   # chunk k: hT[k*128+p, i]
        hT3 = hT[:].rearrange("p (k x) -> p k x", k=KC)
        hT_d3 = hT_d.rearrange("p (k x) -> p k x", k=KC)
        h_all = consts.tile([P, NT * D], bf16)  # tile t: h[t*128+p, d]
        for b in range(NB):
            nc.sync.dma_start(hT3[:, :, ds(b * IB, IB)], hT_d3[:, :, ds(b * IB, IB)])
        for b in range(NB):
            nc.sync.dma_start(
                h_all[:, ds(b * TPB * D, TPB * D)], h_d[:, ds(b * TPB * D, TPB * D)]
            )
        uwu = consts.tile([P, 1], f32)
        nc.gpsimd.dma_start(uwu[:], uwu_d[:])
        # output slab 0 = h passthrough: DRAM->DRAM, streams immediately
        for b in range(NB):
            nc.gpsimd.dma_start(
                out_d[:, ds(b * TPB * D, TPB * D)], h_d[:, ds(b * TPB * D, TPB * D)]
            )
        aux = consts.tile([P, 2 * D], bf16)    # [u | uwbT]
        nc.scalar.dma_start(aux[:], aux_d[:])
        u_sb = aux[:, ds(0, D)]
        uwbT = aux[:, ds(D, D)]
        ident = consts.tile([P, P], bf16)
        make_identity(nc, ident[:])            # gpsimd affine_select

        # ---- constants ----
        warm = consts.tile([P, D], bf16)
        nc.vector.memset(warm[:], 0.25)
        ones_col = consts.tile([P, 1], bf16)
        nc.vector.memset(ones_col[:], 1.0)
        one1 = consts.tile([1, 1], bf16)
        nc.vector.memset(one1[:], 1.0)

        # ---- PE warmup: opens the HAM clock gate while input DMAs fly ----
        wp = ps.tile([P, D], f32, tag="acc", bufs=1)
        for _ in range(5):
            nc.tensor.matmul(wp[:], warm[:, ds(0, P)], warm[:], start=True, stop=True)

        # ---- working tiles ----
        ET = consts.tile([JQ, JX], bf16)
        m_exp = consts.tile([P, NT], f32)
        m_bf = consts.tile([P, NT], bf16)
        z_rec = consts.tile([P, NT], f32)
        hap = ps.tile([1, D], f32, tag="hap", bufs=1)
        ua_blk = [
            stage.tile([P, TPB * D], bf16, tag=f"ua{b}", name=f"ua_blk{b}")
            for b in range(NB)
        ]
        o3_blk = [
            stage.tile([P, TPB * D], bf16, tag=f"o3{b}", name=f"o3_blk{b}")
            for b in range(NB)
        ]
        o4T = consts.tile([P, KC * JX], bf16)

        # ---- scores + exp per block ----
        sps = []
        for b in range(NB):
            sp = ps.tile([JQ, IB], f32, tag="s0")
            for k in range(KC):
                nc.tensor.matmul(
                    sp[:], uwbT[:, ds(k * JQ, JQ)], hT3[:, k, ds(b * IB, IB)],
                    start=(k == 0), stop=(k == KC - 1),
                )
            sps.append(sp)
        for b in range(NB):
            nc.scalar.activation(ET[:, ds(b * IB, IB)], sps[b][:], EXP, bias=uwu[:])

        # ---- per block: ET re-transpose (PE) -> j-max (DVE); zsum via PE ----
        zcol = ps.tile([P, NT], f32, tag="acc", bufs=1)
        for b in range(NB):
            et = ps.tile([P, TPB * P], bf16, tag="tp")
            for q in range(TPB):
                t = b * TPB + q
                nc.tensor.transpose(et[:, ds(q * P, P)], ET[:, ds(t * P, P)], ident[:])
            for q in range(TPB):
                t = b * TPB + q
                nc.tensor.matmul(
                    zcol[:, ds(t, 1)], ET[:, ds(t * P, P)], ones_col[:],
                    start=True, stop=True, skip_group_check=True,
                )
            et3 = et[:].rearrange("p (q x) -> p q x", q=TPB)
            sl = ds(b * TPB, TPB)
            nc.vector.reduce_max(m_exp[:, sl], et3, axis=AX)
            nc.vector.reciprocal(z_rec[:, sl], zcol[:, sl])
            nc.gpsimd.tensor_copy(m_bf[:, sl], m_exp[:, sl])

        # ---- c2q u_a b0; q2c hap; zq; u_a b1; haT ----
        ups = []
        for t in range(TPB):
            up = ps.tile([P, D], f32, tag="ua")
            nc.tensor.matmul(up[:], ET[:, ds(t * P, P)], u_sb, start=True, stop=True)
            ups.append(up)
        for t in range(NT):
            nc.tensor.matmul(
                hap[:], m_bf[:, ds(t, 1)], h_all[:, ds(t * D, D)],
                start=(t == 0), stop=(t == NT - 1), skip_group_check=True,
            )
        mrow = consts.tile([P, 1], f32)
        nc.vector.reduce_sum(mrow[:], m_exp[:], axis=AX)
        mrow_bf = consts.tile([P, 1], bf16)
        nc.gpsimd.tensor_copy(mrow_bf[:], mrow[:])
        zqp = ps.tile([1, 1], f32, tag="acc", bufs=1)
        nc.tensor.matmul(zqp[:], mrow_bf[:], ones_col[:], start=True, stop=True)
        for t in range(TPB, NT):
            up = ps.tile([P, D], f32, tag="ua")
            nc.tensor.matmul(up[:], ET[:, ds(t * P, P)], u_sb, start=True, stop=True)
            ups.append(up)
        rzq = consts.tile([1, 1], f32)
        nc.vector.reciprocal(rzq[:], zqp[:])
        ha_row = consts.tile([1, D], bf16)
        nc.vector.tensor_scalar_mul(ha_row[:], hap[:], rzq[:])
        haT = ps.tile([P, KC], f32, tag="acc", bufs=1)
        for k in range(KC):
            nc.tensor.matmul(
                haT[:, ds(k, 1)], ha_row[:, ds(k * P, P)], one1[:],
                start=True, stop=True, skip_group_check=True,
            )

        # ---- c2q evictions: o2 Scalar x6 + GpSimd x2; o3 on DVE bf16 2x ----
        o2_done = [None] * NT
        for t in range(NT):
            b, q = divmod(t, TPB)
            o2 = ua_blk[b][:, ds(q * D, D)]
            nc.scalar.mul(o2, ups[t][:], z_rec[:, ds(t, 1)])
            o2_done[t] = o2
        for t in range(TPB):
            b, q = divmod(t, TPB)
            nc.vector.tensor_mul(
                o3_blk[b][:, ds(q * D, D)], o2_done[t], h_all[:, ds(t * D, D)]
            )
        nc.sync.dma_start(out_d[:, ds(2 * NT * D, TPB * D)], o3_blk[0][:])
        nc.sync.dma_start(out_d[:, ds(NT * D, TPB * D)], ua_blk[0][:])

        # ---- q2c tail: ha column chunks -> o4T via 4x DVE tensor_scalar ----
        hacol = consts.tile([P, KC], f32)
        nc.vector.tensor_copy(hacol[:], haT[:])
        for k in range(KC):
            nc.vector.tensor_scalar_mul(
                o4T[:, ds(k * JX, JX)], hT[:, ds(k * JX, JX)], hacol[:, ds(k, 1)]
            )
        # ---- o3 b1 + remaining DMAs ----
        for t in range(TPB, NT):
            b, q = divmod(t, TPB)
            nc.vector.tensor_mul(
                o3_blk[b][:, ds(q * D, D)], o2_done[t], h_all[:, ds(t * D, D)]
            )
        nc.gpsimd.dma_start(out_d[:, ds(3 * NT * D, KC * JX)], o4T[:])
        nc.sync.dma_start(out_d[:, ds((NT + TPB) * D, TPB * D)], ua_blk[1][:])
        nc.sync.dma_start(out_d[:, ds((2 * NT + TPB) * D, TPB * D)], o3_blk[1][:])

    nc.compile()
    return nc


def _get_nc():
    if "nc" not in _CACHE:
        _CACHE["nc"] = _build_program()
    return _CACHE["nc"]


def _ensure_axon_hooks_stub():
    import sys
    import types

    try:
        import antenv.axon_hooks  # noqa: F401
    except ImportError:
        mod = types.ModuleType("antenv.axon_hooks")
        _hook = [None]
        mod.set_axon_ntff_profile_hook = lambda hook: _hook.__setitem__(0, hook)
        mod.get_axon_ntff_profile_hook = lambda: _hook[0]
        sys.modules["antenv.axon_hooks"] = mod


def _prep_inputs(h, u, alpha_w):
    """Host-side layout/weight prep (data movement + O(JQ*D) weight folding)."""
    import ml_dtypes

    bf = ml_dtypes.bfloat16
    w_h, w_u, w_hu = alpha_w[:D], alpha_w[D:2 * D], alpha_w[2 * D:]
    in_maps = []
    for n in range(N_B):
        hn = h[n]                                   # [JX, D] f32
        un = u[n]                                   # [JQ, D] f32
        hrows = np.ascontiguousarray(
            hn.reshape(NT, P, D).transpose(1, 0, 2).reshape(P, NT * D)
        ).astype(bf)
        # hT[p, k*JX + i] = h[i, k*128+p]  (chunk-major)
        hT = np.ascontiguousarray(
            hn.T.reshape(KC, P, JX).transpose(1, 0, 2).reshape(P, KC * JX)
        ).astype(bf)
        uwb = un * w_hu[None, :] + w_h[None, :]     # [JQ, D]
        uwbT = uwb.T.reshape(KC, P, JQ).transpose(1, 0, 2).reshape(P, KC * JQ)
        aux = np.concatenate([un, uwbT], axis=1).astype(bf)
        uwu = (un @ w_u).reshape(P, 1).astype(np.float32)
        in_maps.append({"hrows": hrows, "hT": hT, "aux": np.ascontiguousarray(aux),
                        "uwu": uwu})
    return in_maps


def _decode_out(res):
    outs = []
    for n in range(N_B):
        o = np.asarray(res.results[n]["out"]).astype(np.float32)
        slabs = o.reshape(P, 4, NT * D)
        rows = slabs[:, :3, :].reshape(P, 3, NT, D).transpose(2, 0, 1, 3)  # [NT,P,3,D]
        o4 = slabs[:, 3, :].reshape(P, KC, JX).transpose(2, 1, 0)          # [JX,KC,P]
        full = np.concatenate(
            [rows.reshape(JX, 3 * D), o4.reshape(JX, D)], axis=1
        )
        outs.append(full)
    return np.stack(outs, axis=0).reshape(N_B, M_B, JX, 4 * D)


def kernel(h, u, alpha_w, alpha_b=None, **_unused):
    _ensure_axon_hooks_stub()
    from concourse.bass_utils import run_bass_kernel_spmd

    h = np.ascontiguousarray(np.asarray(h, dtype=np.float32)).reshape(N_B, JX, D)
    u = np.ascontiguousarray(np.asarray(u, dtype=np.float32)).reshape(N_B, JQ, D)
    alpha_w = np.ascontiguousarray(np.asarray(alpha_w, dtype=np.float32)).reshape(3 * D)

    nc = _get_nc()
    in_maps = _prep_inputs(h, u, alpha_w)
    res = run_bass_kernel_spmd(nc, in_maps, core_ids=list(range(N_B)))
    return _decode_out(res)


# revision 10
# speedup vs baseline: 1.0943x; 1.0943x over previous
"""Trainium2 Bass kernel for BiDAF-style bidirectional attention (v4).

Reference computation (per batch element n; M=1 folded away):
    s[i,j]  = h[i].w_h + u[j].w_u + (h[i]*u[j]).w_hu + b      [JX, JQ]
    a_u     = softmax_j(s);     u_a[i] = sum_j a_u[i,j] u[j]   (c2q)
    a_h     = softmax_i(max_j s);  h_a = sum_i a_h[i] h[i]     (q2c)
    out     = concat(h, u_a, h*u_a, h*h_a)                     [JX, 4D]

Sharding: data-parallel over batch N=8, one NeuronCore per batch element.
alpha_b drops out (both softmaxes are shift-invariant); accepted but unused.

v4 design notes (on top of v3's bf16-I/O + host-folded weights):
  - Input DMAs chained on the Sync HW queue in priority order (hT-b0 gates
    the first scores; h halves land last).  aux on the Scalar queue, uwu on
    GpSimd.  Total input 2.25MB at the ~300GB/s read path.
  - Output slab 0 (h passthrough) is a pure DRAM->DRAM DMA from the input
    tensor -- it streams during the otherwise write-idle input phase.
  - 5-matmul bf16 warmup => HAM clock gate opens ~11.5us, so scores/c2q/q2c
    matmuls run at 2.4GHz.
  - o2 (= u_a/z PSUM evict) split Scalar ACT x6 / GpSimd tensor_scalar x2;
    o3 = o2*h all-bf16 2x TT on DVE only (DVE+GpSimd share SBUF ports --
    big muls never run on both).  o4 computed transposed from the resident
    hT via 4x-mode DVE tensor_scalar; host un-transposes slab 3.
  - zsum over j via tiny N=1 PE matmuls (ET_t^T @ ones) instead of DVE
    reduces; only the j-max reduces remain on DVE.
"""

import numpy as np

N_B, M_B, JX, JQ, D = 8, 1, 1024, 128, 512
P = 128
NT = JX // P    # 8 i-tiles
KC = D // P     # 4 d-chunks
IB = 512        # i-block width for score matmuls
NB = JX // IB   # 2 blocks
TPB = NT // NB  # 4 tiles per block

_CACHE = {}


def _build_program():
    from contextlib import ExitStack

    import concourse.bass as bass
    import concourse.tile as tile
    from concourse import bacc, mybir
    from concourse.masks import make_identity

    f32 = mybir.dt.float32
    bf16 = mybir.dt.bfloat16
    EXP = mybir.ActivationFunctionType.Exp
    AX = mybir.AxisListType.X
    ds = bass.ds

    nc = bacc.Bacc("TRN2", target_bir_lowering=False, debug=False, num_devices=8)
    h_d = nc.dram_tensor("hrows", [P, NT * D], bf16, kind="ExternalInput").ap()
    hT_d = nc.dram_tensor("hT", [P, KC * JX], bf16, kind="ExternalInput").ap()
    aux_d = nc.dram_tensor("aux", [P, 2 * D], bf16, kind="ExternalInput").ap()
    uwu_d = nc.dram_tensor("uwu", [P, 1], f32, kind="ExternalInput").ap()
    # slabs: 0=h rows, 1=u_a rows, 2=h*u_a rows, 3=o4T chunk-major
    out_d = nc.dram_tensor("out", [P, 4 * NT * D], bf16, kind="ExternalOutput").ap()

    with tile.TileContext(nc) as tc, ExitStack() as ctx:
        consts = ctx.enter_context(tc.tile_pool(name="consts", bufs=1))
        stage = ctx.enter_context(tc.tile_pool(name="stage", bufs=1))
        # PSUM budget (8 banks): acc=1, s0=2, tp=2, ua=2, hap=1
        ps = ctx.enter_context(tc.tile_pool(name="ps", bufs=2, space="PSUM"))

        # ---- input DMAs.  Sync queue carries the big loads in priority
        # order (transfers execute in queue order, so hT-b0 is never
        # starved); aux rides the Scalar queue, uwu the GpSimd queue. ----
        hT = consts.tile([P, KC * JX], bf16)   # chunk k: hT[k*128+p, i]
        hT3 = hT[:].rearrange("p (k x) -> p k x", k=KC)
        hT_d3 = hT_d.rearrange("p (k x) -> p k x", k=KC)
        h_all = consts.tile([P, NT * D], bf16)  # tile t: h[t*128+p, d]
        for b in range(NB):
            nc.sync.dma_start(hT3[:, :, ds(b * IB, IB)], hT_d3[:, :, ds(b * IB, IB)])
        for b in range(NB):
            nc.sync.dma_start(
                h_all[:, ds(b * TPB * D, TPB * D)], h_d[:, ds(b * TPB * D, TPB * D)]
            )
        uwu = consts.tile([P, 1], f32)
        nc.gpsimd.dma_start(uwu[:], uwu_d[:])
        # output slab 0 = h passthrough from SBUF, per half as h lands
        for b in range(NB):
            nc.gpsimd.dma_start(
                out_d[:, ds(b * TPB * D, TPB * D)], h_all[:, ds(b * TPB * D, TPB * D)]
            )
        aux = consts.tile([P, 2 * D], bf16)    # [u | uwbT]
        nc.scalar.dma_start(aux[:], aux_d[:])
        u_sb = aux[:, ds(0, D)]
        uwbT = aux[:, ds(D, D)]
        ident = consts.tile([P, P], bf16)
        make_identity(nc, ident[:])            # gpsimd affine_select

        # ---- constants ----
        warm = consts.tile([P, D], bf16)
        nc.vector.memset(warm[:], 0.25)
        ones_col = consts.tile([P, 1], bf16)
        nc.vector.memset(ones_col[:], 1.0)
        one1 = consts.tile([1, 1], bf16)
        nc.vector.memset(one1[:], 1.0)

        # ---- PE warmup: opens the HAM clock gate while input DMAs fly ----
        wp = ps.tile([P, D], f32, tag="acc", bufs=1)
        for _ in range(5):
            nc.tensor.matmul(wp[:], warm[:, ds(0, P)], warm[:], start=True, stop=True)

        # ---- working tiles ----
        ET = consts.tile([JQ, JX], bf16)
        m_exp = consts.tile([P, NT], f32)
        m_bf = consts.tile([P, NT], bf16)
        z_rec = consts.tile([P, NT], f32)
        hap = ps.tile([1, D], f32, tag="hap", bufs=1)
        ua_blk = [
            stage.tile([P, TPB * D], bf16, tag=f"ua{b}", name=f"ua_blk{b}")
            for b in range(NB)
        ]
        o3_blk = [
            stage.tile([P, TPB * D], bf16, tag=f"o3{b}", name=f"o3_blk{b}")
            for b in range(NB)
        ]
        o4T = consts.tile([P, KC * JX], bf16)

        # ---- scores + exp per block ----
        sps = []
        for b in range(NB):
            sp = ps.tile([JQ, IB], f32, tag="s0")
            for k in range(KC):
                nc.tensor.matmul(
                    sp[:], uwbT[:, ds(k * JQ, JQ)], hT3[:, k, ds(b * IB, IB)],
                    start=(k == 0), stop=(k == KC - 1),
                )
            sps.append(sp)
        for b in range(NB):
            nc.scalar.activation(ET[:, ds(b * IB, IB)], sps[b][:], EXP, bias=uwu[:])

        # ---- per block: ET re-transpose (PE) -> j-max (DVE); zsum via PE ----
        zcol = ps.tile([P, NT], f32, tag="acc", bufs=1)
        for b in range(NB):
            et = ps.tile([P, TPB * P], bf16, tag="tp")
            for q in range(TPB):
                t = b * TPB + q
                nc.tensor.transpose(et[:, ds(q * P, P)], ET[:, ds(t * P, P)], ident[:])
            for q in range(TPB):
                t = b * TPB + q
                nc.tensor.matmul(
                    zcol[:, ds(t, 1)], ET[:, ds(t * P, P)], ones_col[:],
                    start=True, stop=True, skip_group_check=True,
                )
            et3 = et[:].rearrange("p (q x) -> p q x", q=TPB)
            sl = ds(b * TPB, TPB)
            nc.vector.reduce_max(m_exp[:, sl], et3, axis=AX)
            nc.vector.reciprocal(z_rec[:, sl], zcol[:, sl])
            nc.gpsimd.tensor_copy(m_bf[:, sl], m_exp[:, sl])

        # ---- c2q u_a b0; q2c hap; zq; u_a b1; haT ----
        ups = []
        for t in range(TPB):
            up = ps.tile([P, D], f32, tag="ua")
            nc.tensor.matmul(up[:], ET[:, ds(t * P, P)], u_sb, start=True, stop=True)
            ups.append(up)
        for t in range(NT):
            nc.tensor.matmul(
                hap[:], m_bf[:, ds(t, 1)], h_all[:, ds(t * D, D)],
                start=(t == 0), stop=(t == NT - 1), skip_group_check=True,
            )
        mrow = consts.tile([P, 1], f32)
        nc.vector.reduce_sum(mrow[:], m_exp[:], axis=AX)
        mrow_bf = consts.tile([P, 1], bf16)
        nc.gpsimd.tensor_copy(mrow_bf[:], mrow[:])
        zqp = ps.tile([1, 1], f32, tag="acc", bufs=1)
        nc.tensor.matmul(zqp[:], mrow_bf[:], ones_col[:], start=True, stop=True)
        for t in range(TPB, NT):
            up = ps.tile([P, D], f32, tag="ua")
            nc.tensor.matmul(up[:], ET[:, ds(t * P, P)], u_sb, start=True, stop=True)
            ups.append(up)
        rzq = consts.tile([1, 1], f32)
        nc.vector.reciprocal(rzq[:], zqp[:])
        ha_row = consts.tile([1, D], bf16)
        nc.vector.tensor_scalar_mul(ha_row[:], hap[:], rzq[:])
        haT = ps.tile([P, KC], f32, tag="acc", bufs=1)
        for k in range(KC):
            nc.tensor.matmul(
                haT[:, ds(k, 1)], ha_row[:, ds(k * P, P)], one1[:],
                start=True, stop=True, skip_group_check=True,
            )

        # ---- c2q evictions: o2 Scalar x6 + GpSimd x2; o3 on DVE bf16 2x ----
        o2_done = [None] * NT
        for t in range(NT):
            b, q = divmod(t, TPB)
            o2 = ua_blk[b][:, ds(q * D, D)]
            nc.scalar.mul(o2, ups[t][:], z_rec[:, ds(t, 1)])
            o2_done[t] = o2
        for t in range(TPB):
            b, q = divmod(t, TPB)
            nc.vector.tensor_mul(
                o3_blk[b][:, ds(q * D, D)], o2_done[t], h_all[:, ds(t * D, D)]
            )
        nc.sync.dma_start(out_d[:, ds(2 * NT * D, TPB * D)], o3_blk[0][:])
        nc.sync.dma_start(out_d[:, ds(NT * D, TPB * D)], ua_blk[0][:])

        # ---- q2c tail: ha column chunks -> o4T via 4x DVE tensor_scalar ----
        hacol = consts.tile([P, KC], f32)
        nc.vector.tensor_copy(hacol[:], haT[:])
        for k in range(KC):
            nc.vector.tensor_scalar_mul(
                o4T[:, ds(k * JX, JX)], hT[:, ds(k * JX, JX)], hacol[:, ds(k, 1)]
            )
        # ---- o3 b1 + remaining DMAs ----
        for t in range(TPB, NT):
            b, q = divmod(t, TPB)
            nc.vector.tensor_mul(
                o3_blk[b][:, ds(q * D, D)], o2_done[t], h_all[:, ds(t * D, D)]
            )
        nc.gpsimd.dma_start(out_d[:, ds(3 * NT * D, KC * JX)], o4T[:])
        nc.sync.dma_start(out_d[:, ds((NT + TPB) * D, TPB * D)], ua_blk[1][:])
        nc.sync.dma_start(out_d[:, ds((2 * NT + TPB) * D, TPB * D)], o3_blk[1][:])

    nc.compile()
    return nc


def _get_nc():
    if "nc" not in _CACHE:
        _CACHE["nc"] = _build_program()
    return _CACHE["nc"]


def _ensure_axon_hooks_stub():
    import sys
    import types

    try:
        import antenv.axon_hooks  # noqa: F401
    except ImportError:
        mod = types.ModuleType("antenv.axon_hooks")
        _hook = [None]
        mod.set_axon_ntff_profile_hook = lambda hook: _hook.__setitem__(0, hook)
        mod.get_axon_ntff_profile_hook = lambda: _hook[0]
        sys.modules["antenv.axon_hooks"] = mod


def _prep_inputs(h, u, alpha_w):
    """Host-side layout/weight prep (data movement + O(JQ*D) weight folding)."""
    import ml_dtypes

    bf = ml_dtypes.bfloat16
    w_h, w_u, w_hu = alpha_w[:D], alpha_w[D:2 * D], alpha_w[2 * D:]
    in_maps = []
    for n in range(N_B):
        hn = h[n]                                   # [JX, D] f32
        un = u[n]                                   # [JQ, D] f32
        hrows = np.ascontiguousarray(
            hn.reshape(NT, P, D).transpose(1, 0, 2).reshape(P, NT * D)
        ).astype(bf)
        # hT[p, k*JX + i] = h[i, k*128+p]  (chunk-major)
        hT = np.ascontiguousarray(
            hn.T.reshape(KC, P, JX).transpose(1, 0, 2).reshape(P, KC * JX)
        ).astype(bf)
        uwb = un * w_hu[None, :] + w_h[None, :]     # [JQ, D]
        uwbT = uwb.T.reshape(KC, P, JQ).transpose(1, 0, 2).reshape(P, KC * JQ)
        aux = np.concatenate([un, uwbT], axis=1).astype(bf)
        uwu = (un @ w_u).reshape(P, 1).astype(np.float32)
        in_maps.append({"hrows": hrows, "hT": hT, "aux": np.ascontiguousarray(aux),
                        "uwu": uwu})
    return in_maps


def _decode_out(res):
    outs = []
    for n in range(N_B):
        o = np.asarray(res.results[n]["out"]).astype(np.float32)
        slabs = o.reshape(P, 4, NT * D)
        rows = slabs[:, :3, :].reshape(P, 3, NT, D).transpose(2, 0, 1, 3)  # [NT,P,3,D]
        o4 = slabs[:, 3, :].reshape(P, KC, JX).transpose(2, 1, 0)          # [JX,KC,P]
        full = np.concatenate(
            [rows.reshape(JX, 3 * D), o4.reshape(JX, D)], axis=1
        )
        outs.append(full)
    return np.stack(outs, axis=0).reshape(N_B, M_B, JX, 4 * D)


def kernel(h, u, alpha_w, alpha_b=None, **_unused):
    _ensure_axon_hooks_stub()
    from concourse.bass_utils import run_bass_kernel_spmd

    h = np.ascontiguousarray(np.asarray(h, dtype=np.float32)).reshape(N_B, JX, D)
    u = np.ascontiguousarray(np.asarray(u, dtype=np.float32)).reshape(N_B, JQ, D)
    alpha_w = np.ascontiguousarray(np.asarray(alpha_w, dtype=np.float32)).reshape(3 * D)

    nc = _get_nc()
    in_maps = _prep_inputs(h, u, alpha_w)
    res = run_bass_kernel_spmd(nc, in_maps, core_ids=list(range(N_B)))
    return _decode_out(res)
